# revision 45
# baseline (speedup 1.0000x reference)
"""Multi-head causal self-attention (B=4, T=2048, C=1024, H=16) on 8 TRN2 cores.

Sharding: core c handles batch b = c//2 and head-group hg = c%2 (8 heads):
data parallel over B, tensor parallel over H. Each core computes qk^T for its
heads (xT @ Wqk column-slice, transposed per-head-pair layout), V in natural
layout, causal attention for its 8 heads, and a partial output projection
(row-split W_proj) -> y_partial [T, C]. Host transposes x per core and sums
y[b] = y_partial[2b] + y_partial[2b+1] + b_proj.

Single fused schedule: qkv-projection / out-projection / normalization units
are drained as "fillers" between attention inner-loop iterations so the PE
never idles while the Scalar engine works through the softmax exps. Exps are
paired into [128,1024] activations (both heads of a pair share one PSUM
tile spanning 2 banks); attn@V trails scores by one k-block; causal masks
run on the Pool engine (which cannot touch PSUM); PSUM evictions and the
softmax 1/Z path (spread-to-32-partitions + reciprocal_approx_fast +
K<=2 broadcast matmul) run on DVE. Host pre-transposes x/Wqkv/Wv into exact
SBUF layouts so every input DMA moves 1-4KB lines; y is written bf16 and
summed in fp32 on the host. ~292us on 8 cores (baseline: 336us); rel err
~4e-3 (bf16 everywhere except fp32 PSUM accumulation and fp32 1/Z).
"""

from collections import deque
from contextlib import ExitStack
from functools import partial

import ml_dtypes
import numpy as np

import concourse.bass as bass
import concourse.bacc as bacc
import concourse.mybir as mybir
import concourse.tile as tile
from concourse.bass_utils import run_bass_kernel_spmd
from concourse.masks import make_upper_triangular

B, T, C, H, HS = 4, 2048, 1024, 16, 64
P = 128
NQC = T // 512          # q-chunks of 512
NKB = T // P            # key blocks of 128
TH = T // 2             # t-half
SCALE = HS ** -0.5

F32 = mybir.dt.float32
F32R = mybir.dt.float32r
BF16 = mybir.dt.bfloat16
Exp = mybir.ActivationFunctionType.Exp


def _ap3(t, off, width, blk=512):
    """[128, 2, width] strided view: two blocks at stride `blk`, offset `off`."""
    a = t if isinstance(t, bass.AP) else t[:]
    return bass.AP(a.tensor, a.offset + off, [[a.ap[0][0], P], [blk, 2], [1, width]])


def build_kernel():
    nc = bacc.Bacc("TRN2", target_bir_lowering=False)

    xt_d = nc.dram_tensor("xt", (P, 8 * T), BF16, kind="ExternalInput")
    wqk_d = nc.dram_tensor("wqk", (P, 8 * 1024), BF16, kind="ExternalInput")
    bqk_d = nc.dram_tensor("bqk", (8 * P,), F32, kind="ExternalInput")
    wv_d = nc.dram_tensor("wv", (P, 8 * 512), BF16, kind="ExternalInput")
    bv_d = nc.dram_tensor("bv", (1, 512), F32R, kind="ExternalInput")
    wproj_d = nc.dram_tensor("wproj", (8 * HS, C), BF16, kind="ExternalInput")
    y_d = nc.dram_tensor("y", (T, C), BF16, kind="ExternalOutput")

    with tile.TileContext(nc) as tc, ExitStack() as big:
        const = big.enter_context(tc.tile_pool(name="const", bufs=1))
        persist = big.enter_context(tc.tile_pool(name="persist", bufs=1))
        xtp = big.enter_context(tc.tile_pool(name="xtp", bufs=2))
        atp = big.enter_context(tc.tile_pool(name="atp", bufs=3))
        zrrp = big.enter_context(tc.tile_pool(name="zrrp", bufs=2))
        zspp = big.enter_context(tc.tile_pool(name="zspp", bufs=2))
        ysp = big.enter_context(tc.tile_pool(name="ysp", bufs=3))
        ps_s = big.enter_context(tc.tile_pool(name="ps_s", bufs=2, space="PSUM"))
        ps_o = big.enter_context(tc.tile_pool(name="ps_o", bufs=1, space="PSUM"))
        ps_g = big.enter_context(tc.tile_pool(name="ps_g", bufs=2, space="PSUM"))

        # ---------------- persistent SBUF ----------------
        wqk_sb = persist.tile([P, 8 * 1024], BF16, tag="wqk_sb")
        wv_sb = persist.tile([P, 8 * 512], BF16, tag="wv_sb")
        wpj = persist.tile([P, 4 * C], BF16, tag="wpj")
        qk_all = persist.tile([P, 12 * T], BF16, tag="qk")
        v_all = persist.tile([P, 4 * NKB * 130], BF16, tag="v")
        aoT = persist.tile([P, 4 * T], BF16, tag="aoT")
        bqk = persist.tile([P, 8], F32, tag="bqk")
        bias_v = persist.tile([P, 512], F32, tag="bias_v")

        # ---------------- input DMAs (latency-ordered; host pre-transposes
        # xt/wqk/wv into exact SBUF layouts so lines are 1-4KB) ----------------
        # earliest queues get the first matmul group's inputs, interleaved:
        # xt-cb0, wqk00h0, xt-cb1, wqk00h1, xt-cb2, wqk01h0, xt-cb3, wqk01h1,
        # then xt-cb4..7
        xt0a = xtp.tile([P, 8 * 512], BF16, tag="xT0a", bufs=1)
        xt0b = xtp.tile([P, 8 * 512], BF16, tag="xT0b", bufs=1)
        xts = [(xt0a, xt0b)]
        for i in range(4):
            nc.sync.dma_start(
                xt0a[:, i * 512 : (i + 1) * 512],
                xt_d[:, i * T : i * T + 512],
            )
            chb, h = i // 2, i % 2
            nc.sync.dma_start(
                wqk_sb[:, chb * 1024 + h * 512 : chb * 1024 + (h + 1) * 512],
                wqk_d[:, chb * 1024 + h * 512 : chb * 1024 + (h + 1) * 512],
            )
        for cb in range(4, 8):
            nc.sync.dma_start(
                xt0a[:, cb * 512 : (cb + 1) * 512],
                xt_d[:, cb * T : cb * T + 512],
            )
        bvr = const.tile([1, 512], F32R, tag="bvr")
        nc.sync.dma_start(bvr[:], bv_d[:])
        nc.sync.dma_start(bqk[:], bqk_d[:].rearrange("(a p) -> p a", p=P))
        for h in range(4):  # wv in 4 chunks (pv0 needs these early)
            nc.sync.dma_start(
                wv_sb[:, h * 1024 : (h + 1) * 1024],
                wv_d[:, h * 1024 : (h + 1) * 1024],
            )
        for cb in range(8):
            nc.sync.dma_start(
                xt0b[:, cb * 512 : (cb + 1) * 512],
                xt_d[:, cb * T + 512 : cb * T + 1024],
            )
        for chb in range(2, 8):
            for h in range(2):
                nc.sync.dma_start(
                    wqk_sb[:, chb * 1024 + h * 512 : chb * 1024 + (h + 1) * 512],
                    wqk_d[:, chb * 1024 + h * 512 : chb * 1024 + (h + 1) * 512],
                )
        for pr in range(4):
            nc.sync.dma_start(
                wpj[:, pr * C : (pr + 1) * C],
                wproj_d[pr * P : (pr + 1) * P, :],
            )

        # ---------------- constants (Pool engine; off the critical path) ----
        ones_f = const.tile([P, P], F32, tag="ones_f")
        nc.gpsimd.memset(ones_f[:], 1.0)
        ones_t = const.tile([1, P], F32R, tag="ones_t")
        nc.gpsimd.tensor_copy(ones_t[:], ones_f[0:1, :])
        # selab: cols 0:128 select head-A partitions (0:64), cols 128:256 head-B
        selab_f = const.tile([1, 2 * P], F32, tag="selab_f")
        nc.gpsimd.memset(selab_f[:], 0.0)
        nc.gpsimd.tensor_copy(selab_f[0:1, 0:64], ones_f[0:1, 0:64])
        nc.gpsimd.tensor_copy(selab_f[0:1, 192:256], ones_f[0:1, 0:64])
        selab = const.tile([1, 2 * P], F32R, tag="selab")
        nc.gpsimd.tensor_copy(selab[:], selab_f[:])
        # sel01 [2,128]: row0 selects partitions 0:64, row1 selects 64:128
        # (built via DMA: compute engines cannot write partition base 1)
        sel01 = const.tile([2, P], F32R, tag="sel01")
        nc.sync.dma_start(sel01[0:2, :].bitcast(F32), selab_f[0:1, :])
        # mask2[k, q] = 1 where k <= q, two copies side by side
        mask2 = const.tile([P, 2 * P], BF16, tag="mask2")
        make_upper_triangular(nc, mask2[:, 0:P], val=1.0, diag=True)
        make_upper_triangular(nc, mask2[:, P : 2 * P], val=1.0, diag=True)
        # one-time zero pads for the q blocks (pair-ordered: pair p's pads
        # must land before pair p's first scores matmul)
        for p_pair in range(4):
            nc.gpsimd.memset(qk_all[64:P, (3 * p_pair) * T : (3 * p_pair + 1) * T], 0.0)
            nc.gpsimd.memset(
                qk_all[0:64, (3 * p_pair + 1) * T : (3 * p_pair + 2) * T], 0.0
            )
        # v_all ones columns (row-sum trick for Z)
        va4 = v_all[:].rearrange("p (a b c) -> p a b c", a=4, b=NKB, c=130)
        nc.gpsimd.tensor_copy(va4[:, :, :, 64:65], ones_f[:, 0 : 4 * NKB])
        nc.gpsimd.tensor_copy(va4[:, :, :, 129:130], ones_f[:, 0 : 4 * NKB])

        # ---------------- unit emitters ----------------
        def xt_slice(th, cb, t0, width):
            """AP over x^T columns [t0, t0+width) of chunk cb, t-half th."""
            if th == 0:
                tck, off = t0 // 512, t0 % 512
                return xts[0][tck][:, cb * 512 + off : cb * 512 + off + width]
            return xts[1][:, cb * TH + t0 : cb * TH + t0 + width]

        def emit_pq(th, tck, chb):
            p_pair, kind = chb // 2, chb % 2
            pq = ps_g.tile([P, 512], F32, tag="gen", name="pq")
            for cb in range(8):
                nc.tensor.matmul(
                    pq[:],
                    wqk_sb[:, chb * 1024 + cb * P : chb * 1024 + (cb + 1) * P],
                    xt_slice(th, cb, tck * 512, 512),
                    start=(cb == 0),
                    stop=(cb == 7),
                    skip_group_check=True,
                )
            t0 = th * TH + tck * 512
            if kind == 0:  # q -> two zero-padded blocks
                blk_a, blk_b = 3 * p_pair, 3 * p_pair + 1
                nc.vector.tensor_scalar_add(
                    qk_all[0:64, blk_a * T + t0 : blk_a * T + t0 + 512],
                    pq[0:64, :],
                    bqk[0:64, chb : chb + 1],
                )
                nc.vector.tensor_scalar_add(
                    qk_all[64:P, blk_b * T + t0 : blk_b * T + t0 + 512],
                    pq[64:P, :],
                    bqk[64:P, chb : chb + 1],
                )
            else:  # k pair block
                blk = 3 * p_pair + 2
                nc.vector.tensor_scalar_add(
                    qk_all[:, blk * T + t0 : blk * T + t0 + 512],
                    pq[:],
                    bqk[:, chb : chb + 1],
                )

        def emit_pv(kb):
            th, tb = kb // 8, kb % 8
            pv = ps_g.tile([P, 512], F32, tag="gen", name="pv")
            for cb in range(8):
                nc.tensor.matmul(
                    pv[:],
                    xt_slice(th, cb, tb * P, P),
                    wv_sb[:, cb * 512 : (cb + 1) * 512],
                    start=(cb == 0),
                    stop=(cb == 7),
                    skip_group_check=True,
                )
            dst = bass.AP(
                v_all[:].tensor,
                v_all[:].offset + kb * 130,
                [[v_all[:].ap[0][0], P], [NKB * 130, 4], [65, 2], [1, 64]],
            )
            src = bass.AP(
                pv[:].tensor,
                pv[:].offset,
                [[pv[:].ap[0][0], P], [128, 4], [64, 2], [1, 64]],
            )
            bsrc = bass.AP(
                bias_v[:].tensor,
                bias_v[:].offset,
                [[bias_v[:].ap[0][0], P], [128, 4], [64, 2], [1, 64]],
            )
            nc.vector.tensor_tensor(dst, src, bsrc, mybir.AluOpType.add)

        def emit_proj(tb, ys_on_scalar=False):
            ys = ysp.tile([P, 1024], BF16, tag="ys")
            for oc in range(2):
                py = ps_g.tile([P, 512], F32, tag="gen", name="py")
                for pp in range(4):
                    nc.tensor.matmul(
                        py[:],
                        aoT[:, pp * T + tb * P : pp * T + (tb + 1) * P],
                        wpj[:, pp * C + oc * 512 : pp * C + (oc + 1) * 512],
                        start=(pp == 0),
                        stop=(pp == 3),
                        skip_group_check=True,
                    )
                if ys_on_scalar:
                    nc.scalar.copy(ys[:, oc * 512 : (oc + 1) * 512], py[:])
                else:
                    nc.vector.tensor_copy(ys[:, oc * 512 : (oc + 1) * 512], py[:])
            nc.sync.dma_start(y_d[tb * P : (tb + 1) * P, :], ys[:])

        def emit_norm_a(p_pair, zs):
            """PE-free half: spread Z, fast reciprocal, unspread into f32r."""
            zsp = zspp.tile([32, 32], F32, tag=f"zsp{p_pair}", bufs=1)
            nc.sync.dma_start(zsp[0:32, :], zs[0:1, :])
            zspr = zspp.tile([32, 32], F32, tag=f"zspr{p_pair}", bufs=1)
            nc.vector.reciprocal_approx_fast(zspr[0:32, :], zsp[0:32, :])
            rz = zspp.tile([2, 512], F32R, tag=f"rz{p_pair}", bufs=1)
            nc.sync.dma_start(rz[0:1, :].bitcast(F32), zspr[0:16, :])
            nc.sync.dma_start(rz[1:2, :].bitcast(F32), zspr[16:32, :])
            return rz

        def emit_norm_b(qc, p_pair, rz):
            pbt = ps_g.tile([P, 512], F32, tag="gen", name="pbt")
            if rz.partition_size() == 2:  # deferred path: one K=2 matmul
                nc.tensor.matmul(
                    pbt[:], sel01[:], rz[0:2, 0:512],
                    start=True, stop=True, skip_group_check=True,
                )
            else:  # tail path: rz on partition 0 only
                nc.tensor.matmul(
                    pbt[:], selab[0:1, 0:P], rz[0:1, 0:512],
                    start=True, stop=False, skip_group_check=True,
                )
                nc.tensor.matmul(
                    pbt[:], selab[0:1, P : 2 * P], rz[0:1, 512:1024],
                    start=False, stop=True, skip_group_check=True,
                )
            col = p_pair * T + qc * 512
            nc.vector.tensor_mul(
                aoT[:, col : col + 512], aoT[:, col : col + 512], pbt[:]
            )

        fillers = deque()
        pace = {"stride": 1, "i": 0}

        def drain(n=1):
            for _ in range(n):
                if fillers:
                    fillers.popleft()()

        def drain_paced():
            pace["i"] += 1
            if pace["i"] % pace["stride"] == 0:
                drain(1)

        po_last = [None, None]

        def attn_pair(qc, p_pair, skip_z=False, z_on_scalar=False):
            nkb = 4 * qc + 4
            kblk = 3 * p_pair + 2
            vbase = p_pair * NKB * 130
            po0 = ps_o.tile([65, 512], F32, tag="po0")
            po1 = ps_o.tile([65, 512], F32, tag="po1")
            po = (po0, po1)
            po_last[0], po_last[1] = po0, po1
            pend = deque()
            for kb in range(nkb + 1):
                if kb < nkb:
                    qoff = max(0, kb * P - qc * 512)
                    w = 512 - qoff
                    ps = ps_s.tile([P, 1024], F32, tag="ps")
                    for hh in range(2):
                        qblk = 3 * p_pair + hh
                        nc.tensor.matmul(
                            ps[:, hh * 512 + qoff : (hh + 1) * 512],
                            qk_all[:, kblk * T + kb * P : kblk * T + (kb + 1) * P],
                            qk_all[
                                :,
                                qblk * T + qc * 512 + qoff : qblk * T + (qc + 1) * 512,
                            ],
                            start=True,
                            stop=True,
                            skip_group_check=True,
                        )
                    at = atp.tile([P, 1024], BF16, tag="at")
                    if qoff == 0:
                        nc.scalar.activation(at[:], ps[:], Exp, scale=SCALE)
                    else:
                        nc.scalar.activation(
                            _ap3(at, qoff, w), _ap3(ps, qoff, w), Exp, scale=SCALE
                        )
                    if kb * P >= qc * 512:
                        # diagonal block: zero out k > q entries (both heads)
                        nc.gpsimd.tensor_mul(
                            _ap3(at, qoff, P),
                            _ap3(at, qoff, P),
                            _ap3(mask2, 0, P, blk=P),
                        )
                    pend.append((kb, at, qoff))
                if len(pend) > 1 or (kb >= nkb and pend):
                    pkb, pat, pqoff = pend.popleft()
                    for hh in range(2):
                        nc.tensor.matmul(
                            po[hh][:, pqoff:512],
                            v_all[:, vbase + pkb * 130 + hh * 65 : vbase + pkb * 130 + hh * 65 + 65],
                            pat[:, hh * 512 + pqoff : (hh + 1) * 512],
                            start=(pkb == 0),
                            stop=(pkb == nkb - 1),
                            skip_group_check=True,
                        )
                drain_paced()
            # pair tail: cheap Z staging + raw ao eviction (frees the po
            # banks fast; reciprocal/broadcast/normalization all deferred)
            zs = None
            if not skip_z:
                zs = zrrp.tile([1, 1024], F32, tag=f"zs{p_pair}", bufs=1)
                if z_on_scalar:
                    nc.scalar.copy(zs[0:1, 0:512], po0[64:65, :])
                    nc.scalar.copy(zs[0:1, 512:1024], po1[64:65, :])
                else:
                    nc.vector.tensor_copy(zs[0:1, 0:512], po0[64:65, :])
                    nc.vector.tensor_copy(zs[0:1, 512:1024], po1[64:65, :])
            col = p_pair * T + qc * 512
            if z_on_scalar:
                nc.scalar.copy(aoT[0:64, col : col + 512], po0[0:64, :])
                nc.scalar.copy(aoT[64:P, col : col + 512], po1[0:64, :])
            else:
                nc.vector.tensor_copy(aoT[0:64, col : col + 512], po0[0:64, :])
                nc.vector.tensor_copy(aoT[64:P, col : col + 512], po1[0:64, :])
            return zs

        def attn_round(qc, zrs_own_inline=False):
            slots = 4 * (4 * qc + 5)
            n_units = len(fillers) + (3 if zrs_own_inline else 0)
            pace["stride"] = max(1, slots // max(1, n_units))
            pace["i"] = 0
            zrs = []
            for p_pair in range(4):
                if zrs_own_inline and p_pair >= 1:
                    rz = emit_norm_a(p_pair - 1, zrs[p_pair - 1])
                    fillers.append(partial(emit_norm_b, qc, p_pair - 1, rz))
                zrs.append(
                    attn_pair(
                        qc, p_pair,
                        z_on_scalar=(zrs_own_inline and p_pair == 3),
                    )
                )
            return zrs

        # ---------------- schedule ----------------
        # pre-round-0: q,k for pair 0 (t 0:512), bias_v, v block 0
        emit_pq(0, 0, 0)
        emit_pq(0, 0, 1)
        emit_pq(0, 0, 2)
        pbv = ps_g.tile([P, 512], F32, tag="gen", name="pbv")
        nc.tensor.matmul(pbv[:], ones_t[:], bvr[:], start=True, stop=True,
                         skip_group_check=True)
        nc.vector.tensor_copy(bias_v[:], pbv[:])
        emit_pv(0)

        fillers.extend([partial(emit_pv, kb) for kb in (1, 2, 3)])
        fillers.extend([partial(emit_pq, 0, 0, chb) for chb in range(3, 8)])
        fillers.extend([partial(emit_pq, 0, 1, chb) for chb in range(8)])
        fillers.extend([partial(emit_pv, kb) for kb in (4, 5, 6, 7)])
        # th=1 x loads issued up front so they overlap round 0
        xts.append(xtp.tile([P, 8 * TH], BF16, tag="xT", name="xT1", bufs=1))
        for cb in range(8):
            nc.sync.dma_start(
                xts[1][:, cb * TH : (cb + 1) * TH],
                xt_d[:, cb * T + TH : cb * T + 2 * TH],
            )
        zrs0 = attn_round(0)

        rzs0 = [emit_norm_a(p, zrs0[p]) for p in range(4)]
        fillers.extend([partial(emit_pq, 1, 0, chb) for chb in range(8)])
        fillers.extend([partial(emit_norm_b, 0, p, rzs0[p]) for p in range(4)])
        fillers.extend([partial(emit_pv, kb) for kb in (8, 9, 10, 11)])
        fillers.extend([partial(emit_proj, tb) for tb in range(0, 4)])
        zrs1 = attn_round(1)

        # round 2 fillers: norm r1, th1/tck1 proj, v 12..15
        rzs1 = [emit_norm_a(p, zrs1[p]) for p in range(4)]
        fillers.extend([partial(emit_pq, 1, 1, chb) for chb in range(8)])
        fillers.extend([partial(emit_norm_b, 1, p, rzs1[p]) for p in range(4)])
        fillers.extend([partial(emit_pv, kb) for kb in (12, 13, 14, 15)])
        zrs2 = attn_round(2)

        # round 3 fillers: proj r1, norm r2, proj r2; norms of r3 inline
        rzs2 = [emit_norm_a(p, zrs2[p]) for p in range(4)]
        fillers.extend([partial(emit_proj, tb) for tb in range(4, 8)])
        fillers.extend([partial(emit_norm_b, 2, p, rzs2[p]) for p in range(4)])
        zrs3 = attn_round(3, zrs_own_inline=True)

        # tail: last Z chain stays on DVE (no DMA hops), latency covered by
        # the reserved proj units
        drain(len(fillers))
        rzf = zspp.tile([1, 1024], F32, tag="rzf", bufs=1)
        nc.vector.reciprocal_approx_fast(rzf[0:1, :], zrs3[3][0:1, :])
        rz33 = zspp.tile([1, 1024], F32R, tag="rz33", bufs=1)
        nc.gpsimd.tensor_copy(rz33[0:1, :], rzf[0:1, :])
        for tb in (8, 9, 10):
            emit_proj(tb, ys_on_scalar=True)
        emit_norm_b(3, 3, rz33)
        emit_proj(11, ys_on_scalar=True)
        for tb in range(12, 16):
            emit_proj(tb, ys_on_scalar=True)

    nc.compile()
    return nc


def _shard_inputs(x, W_qkv, b_qkv, W_proj):
    """Build the 8 per-core input maps."""
    in_maps = []
    for c in range(8):
        b = c // 2
        hg = c % 2
        heads = [hg * 8 + j for j in range(8)]
        qk_cols = []
        for p in range(4):
            ha, hb = heads[2 * p], heads[2 * p + 1]
            for part in range(2):  # q, k
                qk_cols.extend(range(ha * 192 + part * 64, ha * 192 + part * 64 + 64))
                qk_cols.extend(range(hb * 192 + part * 64, hb * 192 + part * 64 + 64))
        qk_cols = np.array(qk_cols)
        v_cols = []
        for p in range(4):
            ha, hb = heads[2 * p], heads[2 * p + 1]
            v_cols.extend(range(ha * 192 + 128, ha * 192 + 192))
            v_cols.extend(range(hb * 192 + 128, hb * 192 + 192))
        v_cols = np.array(v_cols)
        # pre-transpose into the exact SBUF layouts (2KB+ DMA lines):
        # xt[p, cb*T + t] = x[b][t, cb*128+p]
        xt = x[b].T.reshape(8, 128, T).transpose(1, 0, 2).reshape(128, 8 * T)
        # wqk[p, chb*1024 + cb*128 + j] = W_qkv[cb*128+p, qk_cols[chb*128+j]]
        wqk = (
            W_qkv[:, qk_cols]
            .reshape(8, 128, 8, 128)
            .transpose(1, 2, 0, 3)
            .reshape(128, 8192)
        )
        # wv[p, cb*512 + j] = W_qkv[cb*128+p, v_cols[j]]
        wv = (
            W_qkv[:, v_cols].reshape(8, 128, 512).transpose(1, 0, 2).reshape(128, 4096)
        )
        in_maps.append(
            {
                "xt": np.ascontiguousarray(xt.astype(ml_dtypes.bfloat16)),
                "wqk": np.ascontiguousarray(wqk.astype(ml_dtypes.bfloat16)),
                "bqk": np.ascontiguousarray(b_qkv[qk_cols], dtype=np.float32),
                "wv": np.ascontiguousarray(wv.astype(ml_dtypes.bfloat16)),
                "bv": np.ascontiguousarray(
                    b_qkv[v_cols].reshape(1, 512), dtype=np.float32
                ),
                "wproj": np.ascontiguousarray(
                    W_proj[hg * 512 : (hg + 1) * 512, :].astype(ml_dtypes.bfloat16)
                ),
            }
        )
    return in_maps


_NC = None


def kernel(x, W_qkv, b_qkv, W_proj, b_proj, _trace=False):
    global _NC
    x = np.asarray(x, dtype=np.float32)
    W_qkv = np.asarray(W_qkv, dtype=np.float32)
    b_qkv = np.asarray(b_qkv, dtype=np.float32)
    W_proj = np.asarray(W_proj, dtype=np.float32)
    b_proj = np.asarray(b_proj, dtype=np.float32)

    in_maps = _shard_inputs(x, W_qkv, b_qkv, W_proj)
    if _NC is None:
        _NC = build_kernel()
    res = run_bass_kernel_spmd(
        _NC, in_maps, core_ids=list(range(8)), trace=_trace,
        trace_cores=list(range(8)) if _trace else None,
    )
    out = np.empty((B, T, C), dtype=np.float32)
    for b in range(B):
        out[b] = (
            np.asarray(res.results[2 * b]["y"], dtype=np.float32)
            + np.asarray(res.results[2 * b + 1]["y"], dtype=np.float32)
            + b_proj
        )
    if _trace:
        return out, res
    return out
